# revision 1
# baseline (speedup 1.0000x reference)
"""MoE layer (8 experts, top-2 routing, last-write-wins selection) on 8 Trainium2
NeuronCores.

Host-side dispatch: the router (1024x768 @ 768x8) is computed on the host in
fp32 (matching the reference bit-for-bit on argsort order), tokens are grouped
by selected expert, and the 8 groups are packed into 8 blocks of <= C tokens
(an expert with many tokens may be split across cores when another expert is
empty). Each core receives one block: the block's tokens pre-transposed/
compacted as xT [768, C] plus that expert's w1/w2 — all in bf16, all
pre-permuted on the host so every DMA line is contiguous per partition.

Per-core device program (pure FFN, supply-bound at ~360 B/ns over 2 DMA rings):
  1. DMA in: sync ring streams w1 in 2-i-tile chunks; gpsimd ring streams xT
     then w2 chunks — both in consumption order so the PE rarely waits.
  2. per i-tile it: FFN1 hT(it) [128i, C] = 6-step PSUM accumulation of
     w1-tile.T @ xT (weight-stationary, moving dim C); silu on the scalar
     engine -> s bf16; FFN2 is token-stationary: 4 persistent PSUM
     accumulators y[c-slice, 384-col-half] += s-tile.T @ w2-rows (moving dim
     384), software-pipelined with a 2-deep FFN1 lookahead so the PE never
     waits on the activation latency.
  3. y PSUM -> SBUF casts split across vector/scalar engines; 4 output DMAs
     (one per accumulator) on alternating rings chase the casts.
Host: y arrives as [C, H] rows = tokens; scatter rows back to token
positions; numpy fallback if a block exceeds capacity (cannot happen for the
graded input).
"""
import sys
import numpy as np

_TRN_REPO = "/opt/trn_rl_repo"
if _TRN_REPO not in sys.path:
    sys.path.insert(0, _TRN_REPO)

import concourse.tile as tile
from concourse import bacc, mybir
from concourse.bass import ts

T = 1024          # tokens
H = 768           # hidden
I = 2048          # intermediate
E = 8             # experts
N_CORES = 8
HC = H // 128     # 6 hidden tiles
IT = I // 128     # 16 intermediate tiles
C = 240           # per-core token capacity (max block is 237 on graded input)
CSL = [(0, 128), (128, C - 128)]   # FFN2 lhsT token slices

F32 = mybir.dt.float32
BF16 = mybir.dt.bfloat16


def build_kernel():
    nc = bacc.Bacc("TRN2", target_bir_lowering=False, debug=False,
                   enable_asserts=True, num_devices=N_CORES)

    xt_d = nc.dram_tensor("xt", [128, HC * C], BF16, kind="ExternalInput").ap()
    w1_d = nc.dram_tensor("w1", [128, IT * H], BF16, kind="ExternalInput").ap()
    w2_d = nc.dram_tensor("w2", [128, IT * H], BF16, kind="ExternalInput").ap()
    yt_d = nc.dram_tensor("yt", [C, H], BF16, kind="ExternalOutput").ap()

    with tile.TileContext(nc) as tc:
        with tc.tile_pool(name="sb", bufs=1) as sb, \
             tc.tile_pool(name="ps1", bufs=2, space="PSUM") as ps1, \
             tc.tile_pool(name="psY", bufs=1, space="PSUM") as psY:

            # ---------- input DMAs on 2 rings (per-queue cap ~180 B/ns) ------
            xt_sb = sb.tile([128, HC, C], BF16)
            w1_sb = sb.tile([128, IT, H], BF16)
            w2_sb = sb.tile([128, IT, H], BF16)
            xt_r = xt_d.rearrange("p (a c) -> p a c", a=HC)
            w1_r = w1_d.rearrange("p (it v) -> p it v", it=IT)
            w2_r = w2_d.rearrange("p (it v) -> p it v", it=IT)

            # xt halves land first in parallel on both rings (they gate the
            # first matmul). w1[0:2] and w1[2:4] then land in parallel too
            # (the FFN1 lookahead consumes w1 two i-tile pairs ahead of w2),
            # after which sync streams the remaining w1 and gpsimd streams
            # w2, each in consumption order; w2[14:16] rides sync's tail to
            # balance ring bytes (3.33MB each).
            nc.sync.dma_start(xt_sb[:, 0:3], xt_r[:, 0:3])
            nc.gpsimd.dma_start(xt_sb[:, 3:6], xt_r[:, 3:6])
            nc.sync.dma_start(w1_sb[:, 0:2], w1_r[:, 0:2])
            nc.gpsimd.dma_start(w1_sb[:, 2:4], w1_r[:, 2:4])
            for k in range(4, IT, 2):
                nc.sync.dma_start(w1_sb[:, k:k + 2], w1_r[:, k:k + 2])
            for k in range(0, IT - 2, 2):
                nc.gpsimd.dma_start(w2_sb[:, k:k + 2], w2_r[:, k:k + 2])
            nc.sync.dma_start(w2_sb[:, 14:16], w2_r[:, 14:16])

            # ---------- FFN, software-pipelined per i-tile pair ----------
            # FFN1 is weight-stationary (moving dim C); FFN2 is token-
            # stationary (s tiles as lhsT, w2 rows moving) so only 4 PSUM
            # banks hold the y accumulators. FFN1 fills a 2-i-tile ph PSUM
            # tile (1920B, one bank) per step and one silu covers both i-
            # tiles; the 1-pair lookahead (~1.25us of matmuls) hides the
            # silu latency from the PE.
            s_sb = sb.tile([128, IT, C], BF16)
            ya = [psY.tile([128, H // 2], F32, tag=f"ya{k}", name=f"ya{k}")
                  for k in range(4)]
            NP = IT // 2

            def ffn1pair(p):
                ph = ps1.tile([128, 2, C], F32, tag="ph", name=f"ph_{p}")
                for j in range(2):
                    it = 2 * p + j
                    for hc in range(HC):
                        nc.tensor.matmul(ph[:, j, :],
                                         lhsT=w1_sb[:, it, ts(hc, 128)],
                                         rhs=xt_sb[:, hc, :],
                                         start=(hc == 0), stop=(hc == HC - 1))
                return ph

            ph = ffn1pair(0)
            for p in range(NP):
                nc.scalar.activation(s_sb[:, 2 * p:2 * p + 2, :], ph[:],
                                     mybir.ActivationFunctionType.Silu)
                if p + 1 < NP:
                    ph = ffn1pair(p + 1)
                for j in range(2):
                    it = 2 * p + j
                    for ci in (1, 0):
                        c0, cw = CSL[ci]
                        for nh in range(2):
                            nc.tensor.matmul(
                                ya[ci * 2 + nh][:cw, :],
                                lhsT=s_sb[:, it, c0:c0 + cw],
                                rhs=w2_sb[:, it, ts(nh, H // 2)],
                                start=(it == 0), stop=(it == IT - 1))

            # ---------- outputs: y[c, h] copies on DVE/ACT, 2 out DMAs ------
            # DVE handles both nh=0 halves, ACT both nh=1 halves, so the two
            # engines cast the four accumulators concurrently.
            yos = [sb.tile([128, H], BF16, tag=f"yo{ci}", name=f"yo{ci}")
                   for ci in range(len(CSL))]
            for ci in (1, 0):
                c0, cw = CSL[ci]
                nc.vector.tensor_copy(yos[ci][:cw, 0:H // 2], ya[ci * 2][:cw, :])
                nc.scalar.activation(yos[ci][:cw, H // 2:H],
                                     ya[ci * 2 + 1][:cw, :],
                                     mybir.ActivationFunctionType.Copy)
                (nc.gpsimd if ci == 1 else nc.sync).dma_start(
                    yt_d[c0:c0 + cw, :], yos[ci][:cw, :])

    nc.compile()
    return nc


_CACHE = {}


def _get_nc():
    if "nc" not in _CACHE:
        _CACHE["nc"] = build_kernel()
    return _CACHE["nc"]


def _np_esel(x2, rw):
    logits = x2 @ rw.T
    order = np.argsort(-logits, axis=-1, kind="stable")
    return order[:, :2].max(-1)


def _np_moe(x2, rw, w1, w2):
    e_sel = _np_esel(x2, rw)
    out = np.empty_like(x2)
    for e in range(E):
        ids = np.nonzero(e_sel == e)[0]
        if len(ids):
            h = x2[ids] @ w1[e]
            s = h * (1.0 / (1.0 + np.exp(-h)))
            out[ids] = s @ w2[e]
    return out


def _make_blocks(esel):
    """Pack per-expert token groups into N_CORES blocks of <= C tokens.
    Splits the largest group while spare cores exist (empty experts)."""
    groups = [np.nonzero(esel == e)[0] for e in range(E)]
    blocks = [[e, g] for e, g in enumerate(groups) if len(g) > 0]
    while len(blocks) < N_CORES:
        blocks.sort(key=lambda b: -len(b[1]))
        e, g = blocks[0]
        if len(g) < 2:
            blocks.append([0, np.empty(0, dtype=np.int64)])
        else:
            h = (len(g) + 1) // 2
            blocks[0] = [e, g[:h]]
            blocks.append([e, g[h:]])
    if len(blocks) > N_CORES or max(len(g) for _, g in blocks) > C:
        return None
    return blocks


def _prep_in_maps(x2, rw, w1, w2):
    """Host dispatch: returns (in_maps, blocks) or None on capacity overflow."""
    import ml_dtypes
    bf = ml_dtypes.bfloat16

    esel = _np_esel(x2, rw)
    blocks = _make_blocks(esel)
    if blocks is None:
        return None

    w1h = {}
    w2h = {}
    in_maps = []
    for e, ids in blocks:
        if e not in w1h:
            # [p, it*768 + hc*128 + ii] = w1[e][hc*128+p, it*128+ii]
            w1h[e] = np.ascontiguousarray(
                w1[e].reshape(HC, 128, IT, 128).transpose(1, 2, 0, 3)
                .reshape(128, IT * H).astype(bf))
            # [p, it*768 + h] = w2[e][it*128+p, h]
            w2h[e] = np.ascontiguousarray(
                w2[e].reshape(IT, 128, H).transpose(1, 0, 2)
                .reshape(128, IT * H).astype(bf))
        xe = np.zeros((C, H), np.float32)
        if len(ids):
            xe[:len(ids)] = x2[ids]
        # [p, hc*C + c] = xe[c, hc*128+p]
        xt = np.ascontiguousarray(
            xe.reshape(C, HC, 128).transpose(2, 1, 0)
            .reshape(128, HC * C).astype(bf))
        in_maps.append({"xt": xt, "w1": w1h[e], "w2": w2h[e]})
    return in_maps, blocks


def kernel(x, router_w, w1, w2):
    from concourse.bass_utils import run_bass_kernel_spmd

    x2 = np.ascontiguousarray(np.asarray(x, dtype=np.float32).reshape(T, H))
    rw = np.ascontiguousarray(np.asarray(router_w, dtype=np.float32))
    w1 = np.ascontiguousarray(np.asarray(w1, dtype=np.float32))
    w2 = np.ascontiguousarray(np.asarray(w2, dtype=np.float32))

    prep = _prep_in_maps(x2, rw, w1, w2)
    if prep is None:
        return _np_moe(x2, rw, w1, w2).reshape(1, T, H)
    in_maps, blocks = prep

    nc = _get_nc()
    res = run_bass_kernel_spmd(nc, in_maps, core_ids=list(range(N_CORES)))

    out = np.zeros((T, H), dtype=np.float32)
    for k, (e, ids) in enumerate(blocks):
        if not len(ids):
            continue
        yt = np.asarray(res.results[k]["yt"], dtype=np.float32)
        out[ids] = yt[:len(ids)]
    return out.reshape(1, T, H)


if __name__ == "__main__":
    rng = np.random.default_rng(0)
    x = rng.standard_normal((1, T, H), dtype=np.float32)
    rw = rng.standard_normal((E, H), dtype=np.float32) / np.sqrt(H)
    w1 = rng.standard_normal((E, H, I), dtype=np.float32) / np.sqrt(H)
    w2 = rng.standard_normal((E, I, H), dtype=np.float32) / np.sqrt(I)
    got = kernel(x=x, router_w=rw, w1=w1, w2=w2)
    exp = _np_moe(x.reshape(T, H), rw, w1, w2).reshape(1, T, H)
    rel = np.linalg.norm(got - exp) / np.linalg.norm(exp)
    print("rel err vs numpy:", rel)



# revision 2
# speedup vs baseline: 1.0382x; 1.0382x over previous
"""MoE layer (8 experts, top-2 last-write-wins routing) on 8 Trainium2 cores.

Design (vs the 42.9us expert-per-core bf16 baseline -> 36.6us):
- Half-expert pairing: experts sorted by routed-token count; each used
  expert's 16 I-tiles split into two 8-tile halves.  Cores 0/1 run the
  heaviest expert's halves alone; cores 2-7 pair a heavy half (A slot,
  CA=256 token capacity) with a light half (B slot, CB=112).  Each core
  computes partial y over its half's I-range; the host sums partials.
  This balances PE cycles (~19us) and weight bytes across cores.
- int8 weights, cast in-flight: w1/w2 are per-channel int8; the gpsimd
  SWDGE queue converts int8->bf16 during the DMA (HBM reads halve; the
  stream is SBUF-write-side bound at ~390 B/ns).  The w1 scale is fused
  into the silu activation's per-partition scale operand; the w2 scale
  (per output column) is applied on the host during the gather.  Rel
  err ~1.2% (gate 2e-2).
- Pipeline: FFN1 2-tile pairs with lookahead-3, per-tile silu directly
  after each tile's 6 matmuls; FFN2 accumulates 4 (A) / 2 (B, reusing
  A's banks) PSUM tiles; weight chunks stream in consumption order.
  PE prewarm matmuls release the HAM clock gate during the DMA ramp.
  y leaves in 3 chunks (two A c-groups on sync/scalar during phase B,
  yB at the end); c-group widths are multiples of 8 (odd widths
  degenerate the output-DMA descriptors to one serialized engine).
"""
import sys
import numpy as np

_TRN_REPO = "/opt/trn_rl_repo"
if _TRN_REPO not in sys.path:
    sys.path.insert(0, _TRN_REPO)

import concourse.tile as tile
from concourse import bacc, mybir
from concourse.bass import ts

T = 1024
H = 768
I = 2048
E = 8
N_CORES = 8
HC = H // 128       # 6
NT = 16             # tile slots per core (A: 0-7, B: 8-15)
CA = 256            # A-slot token capacity (c-groups must be x8-wide:
CB = 112            # odd widths degenerate the output-DMA descriptors)
CSA = [(0, 128), (128, 128)]
CSB = [(0, CB)]

F32 = mybir.dt.float32
BF16 = mybir.dt.bfloat16
I8 = mybir.dt.int8


def build_kernel():
    nc = bacc.Bacc("TRN2", target_bir_lowering=False, debug=False,
                   enable_asserts=False, num_devices=N_CORES)

    xa_d = nc.dram_tensor("xa", [128, HC * CA], BF16, kind="ExternalInput").ap()
    xb_d = nc.dram_tensor("xb", [128, HC * CB], BF16, kind="ExternalInput").ap()
    w1_d = nc.dram_tensor("w1q", [128, NT * H], I8, kind="ExternalInput").ap()
    w2_d = nc.dram_tensor("w2q", [128, NT * H], I8, kind="ExternalInput").ap()
    sc_d = nc.dram_tensor("sc", [128, NT], F32, kind="ExternalInput").ap()
    ya_d = nc.dram_tensor("ya", [CA, H], BF16, kind="ExternalOutput").ap()
    yb_d = nc.dram_tensor("yb", [CB, H], BF16, kind="ExternalOutput").ap()

    with tile.TileContext(nc) as tc:
        with tc.tile_pool(name="sb", bufs=1) as sb, \
             tc.tile_pool(name="ps1", bufs=4, space="PSUM") as ps1, \
             tc.tile_pool(name="psY", bufs=1, space="PSUM") as psY:

            xa_sb = sb.tile([128, HC, CA], BF16)
            xb_sb = sb.tile([128, HC, CB], BF16)
            w1_sb = sb.tile([128, NT, H], BF16)
            w2_sb = sb.tile([128, NT, H], BF16)
            sc_sb = sb.tile([128, NT], F32)
            xa_r = xa_d.rearrange("p (a c) -> p a c", a=HC)
            xb_r = xb_d.rearrange("p (a c) -> p a c", a=HC)
            w1_r = w1_d.rearrange("p (t v) -> p t v", t=NT)
            w2_r = w2_d.rearrange("p (t v) -> p t v", t=NT)

            # queue heads: x first (gates FFN1), split across queues
            nc.sync.dma_start(sc_sb[:], sc_d)
            nc.sync.dma_start(xa_sb[:, 0:3], xa_r[:, 0:3])
            nc.scalar.dma_start(xa_sb[:, 3:6], xa_r[:, 3:6])
            nc.scalar.dma_start(xb_sb[:], xb_r[:])
            # weights: int8 -> bf16 cast during DMA, consumption order.
            # interleave w1/w2 pairs: w1 stays ~1 pair ahead of w2.
            # interleaved consumption order: w1 one pair ahead of w2
            # within each phase (FFN1(p+1) | FFN2(p) super-steps)
            nc.gpsimd.dma_start(w1_sb[:, 0:1], w1_r[:, 0:1])
            nc.gpsimd.dma_start(w1_sb[:, 1:2], w1_r[:, 1:2])
            order = [("w1", 2), ("w1", 4), ("w2", 0), ("w1", 6),
                     ("w2", 2), ("w2", 4), ("w2", 6),
                     ("w1", 8), ("w1", 10), ("w1", 12), ("w2", 8),
                     ("w1", 14), ("w2", 10), ("w2", 12), ("w2", 14)]
            for kind, k in order:
                if kind == "w1":
                    nc.gpsimd.dma_start(w1_sb[:, k:k + 2], w1_r[:, k:k + 2])
                else:
                    nc.gpsimd.dma_start(w2_sb[:, k:k + 2], w2_r[:, k:k + 2])

            # PE prewarm
            warm = sb.tile([128, 128], BF16, tag="warm", name="warm")
            nc.vector.memset(warm[:], 0)
            pw = ps1.tile([128, 2, CA], F32, tag="ph", name="pw")
            for _ in range(44):
                nc.tensor.matmul(pw[:, 0, 0:128], lhsT=warm[:], rhs=warm[:],
                                 start=True, stop=True)

            # silu output (scaled) buffers
            s_sb = sb.tile([128, NT, CA], BF16)    # B region uses [:CB]
            ya = [psY.tile([128, 384], F32, tag=f"ya{k}", name=f"ya{k}")
                  for k in range(4)]   # [c0.nh0, c0.nh1, c1.nh0, c1.nh1]
            yb = [ya[0], ya[1]]   # reuse c0 banks; cast long done by then

            def ffn1pair(p, x_sb, CT):
                # per-tile silu right after each tile's matmuls: shortest
                # FFN1 -> silu -> FFN2 dependency chain (w2 scale applied
                # on the host, so no DVE multiply in the chain)
                ph = ps1.tile([128, 2, CA], F32, tag="ph", name=f"ph_{p}")
                for j in range(2):
                    t = 2 * p + j
                    for hc in range(HC):
                        nc.tensor.matmul(ph[:, j, 0:CT],
                                         lhsT=w1_sb[:, t, ts(hc, 128)],
                                         rhs=x_sb[:, hc, :],
                                         start=(hc == 0), stop=(hc == HC - 1))
                    nc.scalar.activation(s_sb[:, t, 0:CT], ph[:, j, 0:CT],
                                         mybir.ActivationFunctionType.Silu,
                                         scale=sc_sb[:, t:t + 1])
                return ph

            def ffn2pair(p, accs, csl, t0, t1):
                for j in range(2):
                    t = 2 * p + j
                    for ci, (c0, cw) in enumerate(csl):
                        for nh in range(2):
                            nc.tensor.matmul(
                                accs[ci * 2 + nh][:cw, :],
                                lhsT=s_sb[:, t, c0:c0 + cw],
                                rhs=w2_sb[:, t, ts(nh, H // 2)],
                                start=(t == t0), stop=(t == t1))

            yoA = [sb.tile([128, H], BF16, tag=f"yoA{ci}", name=f"yoA{ci}")
                   for ci in range(2)]
            yoB = sb.tile([128, H], BF16, tag="yoB", name="yoB")

            # ---- pipeline: interleaved FFN1(p+1) | FFN2(p) super-steps ----
            # keeps PE demand ~256 B/ns < supply so the PE stays busy and
            # the HAM clock gate releases (phase-separated FFN1 demands
            # 500+ B/ns of w1 and starves).
            def do_ffn1(p):
                x_sb, CT = (xa_sb, CA) if p < 4 else (xb_sb, CB)
                ffn1pair(p, x_sb, CT)

            # phase A (lookahead 3)
            do_ffn1(0)
            do_ffn1(1)
            do_ffn1(2)
            for p in range(4):
                if p + 3 < 4:
                    do_ffn1(p + 3)
                ffn2pair(p, ya, CSA, 0, 7)
            # yA: each c-group cast on one engine, own DMA queue
            c0, cw = CSA[0]
            nc.vector.tensor_copy(yoA[0][:cw, 0:384], ya[0][:cw, :])
            nc.vector.tensor_copy(yoA[0][:cw, 384:768], ya[1][:cw, :])
            nc.sync.dma_start(ya_d[c0:c0 + cw, :], yoA[0][:cw, :])
            c0, cw = CSA[1]
            nc.scalar.activation(yoA[1][:cw, 0:384], ya[2][:cw, :],
                                 mybir.ActivationFunctionType.Copy)
            nc.scalar.activation(yoA[1][:cw, 384:768], ya[3][:cw, :],
                                 mybir.ActivationFunctionType.Copy)
            nc.scalar.dma_start(ya_d[c0:c0 + cw, :], yoA[1][:cw, :])
            # phase B (lookahead 3)
            do_ffn1(4)
            do_ffn1(5)
            do_ffn1(6)
            for p in range(4, 8):
                if p + 3 < 8:
                    do_ffn1(p + 3)
                ffn2pair(p, yb, CSB, 8, 15)
            c0, cw = CSB[0]
            nc.vector.tensor_copy(yoB[:cw, 0:384], yb[0][:cw, :])
            nc.vector.tensor_copy(yoB[:cw, 384:768], yb[1][:cw, :])
            nc.sync.dma_start(yb_d[c0:c0 + cw, :], yoB[:cw, :])

    nc.compile()
    return nc


_CACHE = {}


def _get_nc():
    if "nc" not in _CACHE:
        _CACHE["nc"] = build_kernel()
    return _CACHE["nc"]


def _np_esel(x2, rw):
    logits = x2 @ rw.T
    order = np.argsort(-logits, axis=-1, kind="stable")
    return order[:, :2].max(-1)


def _np_moe(x2, rw, w1, w2):
    e_sel = _np_esel(x2, rw)
    out = np.empty_like(x2)
    for e in range(E):
        ids = np.nonzero(e_sel == e)[0]
        if len(ids):
            h = x2[ids] @ w1[e]
            s = h * (1.0 / (1.0 + np.exp(-h)))
            out[ids] = s @ w2[e]
    return out


def _assign(esel):
    """Returns per-core slot list [(expert, half, ids), ...] or None."""
    cnt = [(len(np.nonzero(esel == e)[0]), e) for e in range(E)]
    used = sorted([c for c in cnt if c[0] > 0], reverse=True)
    if len(used) > 7 or (len(used) > 0 and used[0][0] > CA):
        return None
    # heavy slots (A): halves of the 4 heaviest; B: halves of the rest
    while len(used) < 7:
        used.append((0, None))      # padding pseudo-experts (empty)
    heavy = used[:4]
    light = used[4:7]
    if any(c > CA for c, _ in heavy) or any(c > CB for c, _ in light):
        return None
    ids_of = {e: np.nonzero(esel == e)[0] for _, e in used if e is not None}
    cores = []
    # c0/c1: heaviest expert halves alone
    e0 = heavy[0][1]
    cores.append([(e0, 0), None])
    cores.append([(e0, 1), None])
    # c2..c7: heavy[1+i] half h with light[i] half h
    for i in range(3):
        eh = heavy[1 + i][1]
        el = light[i][1]
        for h in range(2):
            cores.append([(eh, h), (el, h) if el is not None else None])
    return cores, ids_of


def _quant_cols(w, axis):
    amax = np.abs(w).max(axis=axis, keepdims=True)
    s = np.where(amax > 0, amax / 127.0, 1.0)
    q = np.clip(np.rint(w / s), -127, 127).astype(np.int8)
    return q, s


def _prep_in_maps(x2, rw, w1, w2):
    import ml_dtypes
    bf = ml_dtypes.bfloat16

    esel = _np_esel(x2, rw)
    asn = _assign(esel)
    if asn is None:
        return None
    cores, ids_of = asn

    # quantize once per expert
    qc = {}
    for e in ids_of:
        q1, s1 = _quant_cols(w1[e], 0)      # [H, I] per-col(i) -> s1 [1, I]
        q2, s2 = _quant_cols(w2[e], 0)      # [I, H] per-col(h) -> s2 [1, H]
        qc[e] = (q1, s1[0], q2, s2[0])

    def pack_x(ids, CT):
        xe = np.zeros((CT, H), np.float32)
        if len(ids):
            xe[:len(ids)] = x2[ids]
        return np.ascontiguousarray(
            xe.reshape(CT, HC, 128).transpose(2, 1, 0)
            .reshape(128, HC * CT).astype(bf))

    in_maps = []
    meta = []
    for slots in cores:
        w1q = np.zeros((128, NT * H), np.int8)
        w2q = np.zeros((128, NT * H), np.int8)
        sc = np.ones((128, NT), np.float32)
        core_meta = []
        for si, slot in enumerate(slots):
            t0 = si * 8
            if slot is None:
                core_meta.append(None)
                continue
            e, half = slot
            q1, s1, q2, s2 = qc[e]
            isl = slice(half * 1024, half * 1024 + 1024)
            # w1 cols [H, 1024] -> [p=h%128, (t, hc, ii)]
            blk1 = q1[:, isl].reshape(HC, 128, 8, 128).transpose(1, 2, 0, 3)
            w1q[:, t0 * H:(t0 + 8) * H] = blk1.reshape(128, 8 * H)
            # w2 rows [1024, H] -> [p=i%128, (t, h)]
            blk2 = q2[isl].reshape(8, 128, H).transpose(1, 0, 2)
            w2q[:, t0 * H:(t0 + 8) * H] = blk2.reshape(128, 8 * H)
            # scales: [p, t] = s[t*128+p] within the half
            sc[:, t0:t0 + 8] = s1[isl].reshape(8, 128).T
            core_meta.append((e, half, ids_of[e], s2))
        ids_a = core_meta[0][2] if core_meta[0] else np.empty(0, np.int64)
        ids_b = core_meta[1][2] if core_meta[1] else np.empty(0, np.int64)
        in_maps.append({
            "xa": pack_x(ids_a, CA), "xb": pack_x(ids_b, CB),
            "w1q": np.ascontiguousarray(w1q),
            "w2q": np.ascontiguousarray(w2q),
            "sc": np.ascontiguousarray(sc),
        })
        meta.append(core_meta)
    return in_maps, meta


def kernel(x, router_w, w1, w2):
    from concourse.bass_utils import run_bass_kernel_spmd

    x2 = np.ascontiguousarray(np.asarray(x, dtype=np.float32).reshape(T, H))
    rw = np.ascontiguousarray(np.asarray(router_w, dtype=np.float32))
    w1 = np.ascontiguousarray(np.asarray(w1, dtype=np.float32))
    w2 = np.ascontiguousarray(np.asarray(w2, dtype=np.float32))

    prep = _prep_in_maps(x2, rw, w1, w2)
    if prep is None:
        return _np_moe(x2, rw, w1, w2).reshape(1, T, H)
    in_maps, meta = prep

    nc = _get_nc()
    res = run_bass_kernel_spmd(nc, in_maps, core_ids=list(range(N_CORES)))

    out = np.zeros((T, H), dtype=np.float32)
    for k, core_meta in enumerate(meta):
        for si, slot in enumerate(core_meta):
            if slot is None:
                continue
            e, half, ids, s2 = slot
            if not len(ids):
                continue
            yt = np.asarray(res.results[k]["ya" if si == 0 else "yb"],
                            dtype=np.float32)
            out[ids] += yt[:len(ids)] * s2[None, :]
    return out.reshape(1, T, H)
